# revision 14
# baseline (speedup 1.0000x reference)
"""Compressed Interaction Network (CIN) kernel for Trainium2, 8 NeuronCores.

Reference computation (per layer l with weights W[F0, Fk, S], bias b[S]):
    z[b,s,d] = relu( sum_{h,k} x0[b,h,d] * xk[b,k,d] * W[h,k,s] + b[s] )
    split_half: xk_next = z[:, :S/2, :], direct_l = z[:, S/2:, :] (last: all)
    out = sum_d concat(direct_0, direct_1, direct_2)    # [B, 64+64+128]

Strategy (v3):
  - Data parallel over batch: each of 8 cores gets B/8 = 256 batches; per
    core work in "transposed" layout [field, bd], bd = b*16 + d (BD = 4096
    columns), tiled into 8 column tiles of NT=512.
  - Product tensor p[(h,k), bd] = x0[h,bd] * xk[k,bd]: the replicated x0
    factor is materialized host-side (pure data movement) and streamed from
    HBM as one large contiguous DMA per tile.
  - Layer 0 (symmetric, 780 h<=k rows, folded weights): BOTH factors stream
    from HBM; the multiply happens inside the DMA engines - plain load of
    factor A, then a SWDGE DMA with accum_op=mult streams factor B over it.
    No vector-engine work at all.
  - Layers 1/2 share one replication array rep12 (x0[h] repeated 64x); the
    xk-side stack [xk; xk] is built on device (ScalarE writes relu'd z[0:64],
    one SBUF->SBUF DMA copies to partitions 64..127). The multiply runs as
    ONE wide DVE tensor_tensor (fp16 2x mode) over chunks 0..16 plus 3
    GpSimd singles (17..19) - few ops amortize the ~1us DVE pipe drain.
    Layer 2's wide multiply is in-place over the rep tile (its last reader).
  - Matmuls accumulate z[s, bd] in PSUM over hk chunks in fp16 (1 cyc/row);
    ScalarE applies bias+relu; d-reductions on DVE (L0+L1 fused into one).
  - 3 column-tile streams interleaved at chunk granularity.
"""
import numpy as np

import concourse.bass as bass
import concourse.mybir as mybir
from concourse.tile import TileContext
from concourse.bass_utils import run_bass_kernel_spmd

F32 = mybir.dt.float32
F16 = mybir.dt.float16
MULT = mybir.AluOpType.mult
ADD = mybir.AluOpType.add
RELU = mybir.ActivationFunctionType.Relu
AXX = mybir.AxisListType.X

N_CORES = 8
B, F0, D = 2048, 39, 16
S = 128                    # layer size
BC = B // N_CORES          # 256 batches per core
BD = BC * D                # 4096 columns per core
NT = 512                   # bd-tile width
TILES = BD // NT           # 8
NP0 = F0 * (F0 + 1) // 2   # 780 symmetric (h<=k) pairs for layer 0
L0_CH = 7                  # ceil(780/128); last chunk has 12 rows
L0_LAST = NP0 - (L0_CH - 1) * 128   # 12
L12_CH = 20                # ceil(2496/128); last chunk has 64 rows
WIDE = 17                  # chunks 0..16 in one DVE multiply; rest GpSimd
GROUP = 3                  # interleaved column-tile streams

MAX_WAITS = 1


def _fix_sync_overflow(nc):
    """This walrus build accepts at most one semaphore wait per instruction;
    Tile can attach several. Hoist extras onto NoOps spliced right before the
    offending instruction on the same engine (same-engine order is
    sequential, so earlier waits are equivalent). Updates stay put."""
    n_new = 0
    for blk in nc.main_func.blocks:
        out = []
        changed = False
        for inst in blk.instructions:
            si = inst.sync_info
            waits = list(si.on_wait) if si is not None else []
            if len(waits) > MAX_WAITS:
                changed = True
                extra, keep = waits[:-MAX_WAITS], waits[-MAX_WAITS:]
                for i in range(0, len(extra), MAX_WAITS):
                    nop = mybir.InstNoOp(name=f"wsplit-{n_new}", ins=[], outs=[])
                    n_new += 1
                    nop.engine = inst.engine
                    nop.sync_info = mybir.SyncInfo(
                        on_wait=extra[i:i + MAX_WAITS], on_update=[])
                    nc.register_instruction(nop, overwrite=True)
                    out.append(nop)
                si.on_wait = keep
            out.append(inst)
        if changed:
            blk.instructions = out
    return n_new


def _build_kernel():
    nc = bass.Bass(trn_type="TRN2")

    Ad = nc.dram_tensor("Ad", [128, TILES * L0_CH * NT], F16,
                        kind="ExternalInput")
    Bd = nc.dram_tensor("Bd", [128, TILES * L0_CH * NT], F16,
                        kind="ExternalInput")
    repd = nc.dram_tensor("repd", [128, TILES * L12_CH * NT], F16,
                          kind="ExternalInput")
    w0d = nc.dram_tensor("w0d", [128, L0_CH * S], F16, kind="ExternalInput")
    w1d = nc.dram_tensor("w1d", [128, L12_CH * S], F16, kind="ExternalInput")
    w2d = nc.dram_tensor("w2d", [128, L12_CH * S], F16, kind="ExternalInput")
    biasd = nc.dram_tensor("biasd", [128, 5], F32, kind="ExternalInput")
    e2d = nc.dram_tensor("e2d", [64, S], F16, kind="ExternalInput")
    y = nc.dram_tensor("y", [2 * S, BC], F32, kind="ExternalOutput")

    with TileContext(nc) as tc:
        with tc.tile_pool(name="static", bufs=1) as st, \
             tc.tile_pool(name="ab", bufs=3) as ab, \
             tc.tile_pool(name="rep", bufs=4) as rp, \
             tc.tile_pool(name="pwide", bufs=2) as pw, \
             tc.tile_pool(name="ps", bufs=3) as psn, \
             tc.tile_pool(name="tmp", bufs=3) as tp, \
             tc.tile_pool(name="zps", bufs=6, space="PSUM") as zp, \
             tc.tile_pool(name="stk", bufs=2, space="PSUM") as sp:

            # ---- static tiles -------------------------------------------
            w0s = st.tile([128, L0_CH * S], F16)
            w1s = st.tile([128, L12_CH * S], F16)
            w2s = st.tile([128, L12_CH * S], F16)
            bias_s = st.tile([128, 5], F32)
            xk1s = st.tile([128, BD], F16)
            xk2s = st.tile([128, BD], F16)
            o01s = st.tile([S, 2 * BC], F32)     # cols (l, b): direct0|direct1
            o2s = st.tile([S, BC], F32)
            e2s = st.tile([64, S], F16)

            nc.scalar.dma_start(w0s[:, :], w0d[:, :])
            nc.scalar.dma_start(w1s[:, :], w1d[:, :])
            nc.scalar.dma_start(w2s[:, :], w2d[:, :])
            nc.scalar.dma_start(bias_s[:, :], biasd[:, :])
            nc.scalar.dma_start(e2s[:, :], e2d[:, :])

            def make_stack(xkdst, ts):
                """xkdst[64:128, ts] = xkdst[0:64, ts] without a DMA: PE
                identity-pair matmul into PSUM, ScalarE copies the top half
                back to SBUF (engines are lane-locked; PE crosses partitions
                cheaper than the SBUF->SBUF DMA's fixed latency)."""
                stkps = sp.tile([S, NT], F32, tag="stk")
                nc.tensor.matmul(stkps[:, :], e2s[:, :], xkdst[0:64, ts],
                                 start=True, stop=True)
                nc.scalar.copy(xkdst[64:128, ts], stkps[64:128, :])

            def layer12(t, repti, xksrc, xknext, ws, tmp01, zcol_dup,
                        zcol_nat):
                """One of layers 1/2 for column tile t. Yields between
                matmuls so streams interleave. Layer 2 (xknext None)
                multiplies in place over the rep tile."""
                ts = bass.ts(t, NT)
                zps = zp.tile([S, NT], F32, tag="z")
                src0 = xksrc[:, ts].unsqueeze(1)
                if xknext is not None:
                    pws = pw.tile([128, WIDE * NT], F16, tag="pw")
                    pdst = pws[:, :]
                else:
                    pdst = repti[:, 0:WIDE * NT]    # in-place: last reader
                nc.vector.tensor_tensor(
                    pdst.rearrange("p (c n) -> p c n", n=NT),
                    src0.to_broadcast((128, WIDE, NT)),
                    repti[:, 0:WIDE * NT].rearrange("p (c n) -> p c n", n=NT),
                    op=MULT)
                ptiles = {c: pdst[:, c * NT:(c + 1) * NT]
                          for c in range(WIDE)}
                ntail = L12_CH - WIDE
                p3 = psn.tile([128, ntail * NT], F16, tag="p3")
                nc.gpsimd.tensor_tensor(
                    p3[:, :].rearrange("p (c n) -> p c n", n=NT),
                    src0.to_broadcast((128, ntail, NT)),
                    repti[:, WIDE * NT:L12_CH * NT]
                    .rearrange("p (c n) -> p c n", n=NT),
                    op=MULT)
                for c in range(WIDE, L12_CH):
                    ptiles[c] = p3[:, (c - WIDE) * NT:(c - WIDE + 1) * NT]
                yield
                for c in range(L12_CH):
                    part = 128 if c < L12_CH - 1 else 64
                    nc.tensor.matmul(zps[:, :], ws[:part, bass.ts(c, S)],
                                     ptiles[c][:part, :],
                                     start=(c == 0), stop=(c == L12_CH - 1))
                    if c % 4 == 3:
                        yield
                # epilogue
                if xknext is not None:     # layer 1
                    nc.scalar.activation(
                        xknext[0:64, ts], zps[0:64, :], RELU,
                        bias=bias_s[0:64, zcol_dup:zcol_dup + 1])
                    make_stack(xknext, ts)
                    nc.scalar.activation(
                        tmp01[64:S, NT:2 * NT], zps[64:S, :], RELU,
                        bias=bias_s[64:S, zcol_nat:zcol_nat + 1])
                    # fused d-reduction for direct0 and direct1
                    nc.vector.tensor_reduce(
                        o01s[64:S, :].rearrange("p (l q) -> p l q", l=2)
                        [:, :, bass.ts(t, NT // D)],
                        tmp01[64:S, :].rearrange(
                            "p (l b d) -> p l b d", l=2, d=D),
                        axis=AXX, op=ADD)
                else:                      # layer 2
                    tmp = tp.tile([S, NT], F16, tag="tmp2")
                    nc.scalar.activation(
                        tmp[:, :], zps[:, :], RELU,
                        bias=bias_s[:, zcol_nat:zcol_nat + 1])
                    nc.vector.tensor_reduce(
                        o2s[:, bass.ts(t, NT // D)],
                        tmp[:, :].rearrange("p (b d) -> p b d", d=D),
                        axis=AXX, op=ADD)
                yield

            def stream(t):
                ts = bass.ts(t, NT)
                # streaming loads (SP HWDGE ring, one large DMA each)
                Ati = ab.tile([128, L0_CH * NT], F16, tag="A")
                Bti = ab.tile([128, L0_CH * NT], F16, tag="B")
                nc.sync.dma_start(Ati[:, :], Ad[:, bass.ts(t, L0_CH * NT)])
                nc.sync.dma_start(Bti[:, :], Bd[:, bass.ts(t, L0_CH * NT)])
                yield
                # ---- layer 0: one wide multiply, in-place over A --------
                zps = zp.tile([S, NT], F32, tag="z")
                nc.vector.tensor_tensor(Ati[:, :], Ati[:, :], Bti[:, :],
                                        op=MULT)
                repti = rp.tile([128, L12_CH * NT], F16, tag="rep")
                nc.sync.dma_start(repti[:, :],
                                  repd[:, bass.ts(t, L12_CH * NT)])
                yield
                for c in range(L0_CH):
                    part = 128 if c < L0_CH - 1 else L0_LAST
                    nc.tensor.matmul(zps[:, :], w0s[:part, bass.ts(c, S)],
                                     Ati[:part, bass.ts(c, NT)],
                                     start=(c == 0), stop=(c == L0_CH - 1))
                    if c % 4 == 3:
                        yield
                # epilogue L0
                tmp01 = tp.tile([S, 2 * NT], F16, tag="tmp01")
                nc.scalar.activation(xk1s[0:64, ts], zps[0:64, :], RELU,
                                     bias=bias_s[0:64, 0:1])
                make_stack(xk1s, ts)
                nc.scalar.activation(tmp01[64:S, 0:NT], zps[64:S, :], RELU,
                                     bias=bias_s[64:S, 1:2])
                yield
                yield from layer12(t, repti, xk1s, xk2s, w1s, tmp01, 2, 3)
                yield from layer12(t, repti, xk2s, None, w2s, tmp01, 4, 4)

            pending = list(range(TILES))
            gens = []
            while gens or pending:
                while len(gens) < GROUP and pending:
                    gens.append(stream(pending.pop(0)))
                for gen in list(gens):
                    try:
                        next(gen)
                    except StopIteration:
                        gens.remove(gen)

            nc.scalar.dma_start(y[0:64, :], o01s[64:S, 0:BC])
            nc.scalar.dma_start(y[64:S, :], o01s[64:S, BC:2 * BC])
            nc.scalar.dma_start(y[S:2 * S, :], o2s[:, :])

    _fix_sync_overflow(nc)
    return nc


_NC_CACHE = None


def _get_nc():
    global _NC_CACHE
    if _NC_CACHE is None:
        _NC_CACHE = _build_kernel()
    return _NC_CACHE


# symmetric (h<=k) pair index arrays for layer 0
_HH = np.concatenate([np.full(F0 - h, h, np.int64) for h in range(F0)])
_KK = np.concatenate([np.arange(h, F0) for h in range(F0)])


def _pack_cols(M, nch):
    """[rows<=nch*128, BD] -> [128, TILES*nch*NT]: per column tile t, chunk
    c lives at cols [t*nch*NT + c*NT : ... + NT], partition p = row c*128+p."""
    P = np.zeros((nch * 128, BD), np.float16)
    P[:M.shape[0]] = M
    P = P.reshape(nch, 128, TILES, NT).transpose(1, 2, 0, 3)
    return np.ascontiguousarray(P.reshape(128, TILES * nch * NT))


def _pack_w(Wr, nch):
    """[rows<=nch*128, S] -> [128, nch*S] fp16 chunk-major."""
    P = np.zeros((nch * 128, S), np.float16)
    P[:Wr.shape[0]] = Wr
    return np.ascontiguousarray(
        P.reshape(nch, 128, S).transpose(1, 0, 2).reshape(128, nch * S))


def _prep_shared(w_list, b_list):
    """Weight/bias packing shared by all cores."""
    w0f, w1f, w2f = [np.asarray(w, np.float32) for w in w_list]
    w0sym = w0f[_HH, _KK] + np.where((_HH != _KK)[:, None],
                                     w0f[_KK, _HH], 0.0)   # [780, S]
    w0p = _pack_w(w0sym.astype(np.float16), L0_CH)
    w1p = _pack_w(w1f.reshape(F0 * 64, S).astype(np.float16), L12_CH)
    w2p = _pack_w(w2f.reshape(F0 * 64, S).astype(np.float16), L12_CH)
    b0, b1, b2 = [np.asarray(b, np.float32) for b in b_list]
    biases = np.stack([
        np.concatenate([b0[:64], b0[:64]]), b0,
        np.concatenate([b1[:64], b1[:64]]), b1, b2],
        axis=1).astype(np.float32)          # [128, 5]
    e2 = np.zeros((64, S), np.float16)
    e2[np.arange(64), np.arange(64)] = 1.0
    e2[np.arange(64), 64 + np.arange(64)] = 1.0
    return {"w0d": w0p, "w1d": w1p, "w2d": w2p, "biasd": biases, "e2d": e2}


def _prep_core_inputs(inputs, shared, core):
    """Host-side layout prep for one core's batch slice (data movement
    only: transpose, gather, repeat — no arithmetic)."""
    xs = inputs[core * BC:(core + 1) * BC]          # [BC, F0, D]
    x0t = np.ascontiguousarray(
        xs.transpose(1, 0, 2).reshape(F0, BD)).astype(np.float16)
    A = _pack_cols(x0t[_HH], L0_CH)
    Bm = _pack_cols(x0t[_KK], L0_CH)
    rep = _pack_cols(np.repeat(x0t, 64, axis=0), L12_CH)
    return {"Ad": A, "Bd": Bm, "repd": rep, **shared}


def kernel(inputs, w0, w1, w2, b0, b1, b2, _trace=False):
    inputs = np.asarray(inputs, np.float32)
    shared = _prep_shared((w0, w1, w2), (b0, b1, b2))

    nc = _get_nc()
    in_maps = [_prep_core_inputs(inputs, shared, core)
               for core in range(N_CORES)]
    res = run_bass_kernel_spmd(nc, in_maps, core_ids=list(range(N_CORES)),
                               trace=_trace)
    outs = []
    for core in range(N_CORES):
        yc = res.results[core]["y"]          # [256 s_cat, 256 b]
        outs.append(np.ascontiguousarray(yc.T))
    full = np.concatenate(outs, axis=0)       # [2048, 256]
    if _trace:
        return full, res
    return full


# revision 16
# speedup vs baseline: 1.1061x; 1.1061x over previous
"""Compressed Interaction Network (CIN) kernel for Trainium2, 8 NeuronCores.

Reference computation (per layer l with weights W[F0, Fk, S], bias b[S]):
    z[b,s,d] = relu( sum_{h,k} x0[b,h,d] * xk[b,k,d] * W[h,k,s] + b[s] )
    split_half: xk_next = z[:, :S/2, :], direct_l = z[:, S/2:, :] (last: all)
    out = sum_d concat(direct_0, direct_1, direct_2)    # [B, 64+64+128]

Strategy (v3):
  - Data parallel over batch: each of 8 cores gets B/8 = 256 batches; per
    core work in "transposed" layout [field, bd], bd = b*16 + d (BD = 4096
    columns), tiled into 8 column tiles of NT=512.
  - Product tensor p[(h,k), bd] = x0[h,bd] * xk[k,bd]: the replicated x0
    factor is materialized host-side (pure data movement) and streamed from
    HBM as one large contiguous DMA per tile.
  - Layer 0 (symmetric, 780 h<=k rows, folded weights): BOTH factors stream
    from HBM; the multiply happens inside the DMA engines - plain load of
    factor A, then a SWDGE DMA with accum_op=mult streams factor B over it.
    No vector-engine work at all.
  - Layers 1/2 share one replication array rep12 (x0[h] repeated 64x); the
    xk-side stack [xk; xk] is built on device (ScalarE writes relu'd z[0:64],
    one SBUF->SBUF DMA copies to partitions 64..127). The multiply runs as
    ONE wide DVE tensor_tensor (fp16 2x mode) over chunks 0..16 plus 3
    GpSimd singles (17..19) - few ops amortize the ~1us DVE pipe drain.
    Layer 2's wide multiply is in-place over the rep tile (its last reader).
  - Matmuls accumulate z[s, bd] in PSUM over hk chunks in fp16 (1 cyc/row);
    ScalarE applies bias+relu; d-reductions on DVE (L0+L1 fused into one).
  - 3 column-tile streams interleaved at chunk granularity.
"""
import numpy as np

import concourse.bass as bass
import concourse.mybir as mybir
from concourse.tile import TileContext
from concourse.bass_utils import run_bass_kernel_spmd

F32 = mybir.dt.float32
F16 = mybir.dt.float16
MULT = mybir.AluOpType.mult
ADD = mybir.AluOpType.add
RELU = mybir.ActivationFunctionType.Relu
AXX = mybir.AxisListType.X

N_CORES = 8
B, F0, D = 2048, 39, 16
S = 128                    # layer size
BC = B // N_CORES          # 256 batches per core
BD = BC * D                # 4096 columns per core
NT = 512                   # bd-tile width
TILES = BD // NT           # 8
NP0 = F0 * (F0 + 1) // 2   # 780 symmetric (h<=k) pairs for layer 0
L0_CH = 7                  # ceil(780/128); last chunk has 12 rows
L0_LAST = NP0 - (L0_CH - 1) * 128   # 12
L12_CH = 20                # ceil(2496/128); last chunk has 64 rows
WIDE = 18                  # chunks 0..17 in one DVE multiply; rest GpSimd
GROUP = 3                  # interleaved column-tile streams

MAX_WAITS = 1


def _fix_sync_overflow(nc):
    """This walrus build accepts at most one semaphore wait per instruction;
    Tile can attach several. Hoist extras onto NoOps spliced right before the
    offending instruction on the same engine (same-engine order is
    sequential, so earlier waits are equivalent). Updates stay put."""
    n_new = 0
    for blk in nc.main_func.blocks:
        out = []
        changed = False
        for inst in blk.instructions:
            si = inst.sync_info
            waits = list(si.on_wait) if si is not None else []
            if len(waits) > MAX_WAITS:
                changed = True
                extra, keep = waits[:-MAX_WAITS], waits[-MAX_WAITS:]
                for i in range(0, len(extra), MAX_WAITS):
                    nop = mybir.InstNoOp(name=f"wsplit-{n_new}", ins=[], outs=[])
                    n_new += 1
                    nop.engine = inst.engine
                    nop.sync_info = mybir.SyncInfo(
                        on_wait=extra[i:i + MAX_WAITS], on_update=[])
                    nc.register_instruction(nop, overwrite=True)
                    out.append(nop)
                si.on_wait = keep
            out.append(inst)
        if changed:
            blk.instructions = out
    return n_new


def _build_kernel():
    nc = bass.Bass(trn_type="TRN2")

    Ad = nc.dram_tensor("Ad", [128, TILES * L0_CH * NT], F16,
                        kind="ExternalInput")
    Bd = nc.dram_tensor("Bd", [128, TILES * L0_CH * NT], F16,
                        kind="ExternalInput")
    repd = nc.dram_tensor("repd", [128, TILES * L12_CH * NT], F16,
                          kind="ExternalInput")
    w0d = nc.dram_tensor("w0d", [128, L0_CH * S], F16, kind="ExternalInput")
    w1d = nc.dram_tensor("w1d", [128, L12_CH * S], F16, kind="ExternalInput")
    w2d = nc.dram_tensor("w2d", [128, L12_CH * S], F16, kind="ExternalInput")
    biasd = nc.dram_tensor("biasd", [128, 5], F32, kind="ExternalInput")
    e2d = nc.dram_tensor("e2d", [64, S], F16, kind="ExternalInput")
    y = nc.dram_tensor("y", [2 * S, BC], F32, kind="ExternalOutput")

    with TileContext(nc) as tc:
        with tc.tile_pool(name="static", bufs=1) as st, \
             tc.tile_pool(name="ab", bufs=3) as ab, \
             tc.tile_pool(name="rep", bufs=4) as rp, \
             tc.tile_pool(name="pwide", bufs=2) as pw, \
             tc.tile_pool(name="ps", bufs=3) as psn, \
             tc.tile_pool(name="tmp", bufs=2) as tp, \
             tc.tile_pool(name="zps", bufs=6, space="PSUM") as zp, \
             tc.tile_pool(name="stk", bufs=2, space="PSUM") as sp:

            # ---- static tiles -------------------------------------------
            w0s = st.tile([128, L0_CH * S], F16)
            w1s = st.tile([128, L12_CH * S], F16)
            w2s = st.tile([128, L12_CH * S], F16)
            bias_s = st.tile([128, 5], F32)
            xk1s = st.tile([128, BD], F16)
            xk2s = st.tile([128, BD], F16)
            o01s = st.tile([S, 2 * BC], F32)     # cols (l, b): direct0|direct1
            o2s = st.tile([S, BC], F32)
            e2s = st.tile([64, S], F16)

            nc.scalar.dma_start(w0s[:, :], w0d[:, :])
            nc.scalar.dma_start(w1s[:, :], w1d[:, :])
            nc.scalar.dma_start(w2s[:, :], w2d[:, :])
            nc.scalar.dma_start(bias_s[:, :], biasd[:, :])
            nc.scalar.dma_start(e2s[:, :], e2d[:, :])

            def make_stack(xkdst, ts):
                """xkdst[64:128, ts] = xkdst[0:64, ts] without a DMA: PE
                identity-pair matmul into PSUM, ScalarE copies the top half
                back to SBUF (engines are lane-locked; PE crosses partitions
                cheaper than the SBUF->SBUF DMA's fixed latency)."""
                stkps = sp.tile([S, NT], F32, tag="stk")
                nc.tensor.matmul(stkps[:, :], e2s[:, :], xkdst[0:64, ts],
                                 start=True, stop=True)
                nc.scalar.copy(xkdst[64:128, ts], stkps[64:128, :])

            def layer12(t, repti, xksrc, xknext, ws, tmp01, tmp2, zcol_dup,
                        zcol_nat):
                """One of layers 1/2 for column tile t. Yields between
                matmuls so streams interleave. Layer 2 (xknext None)
                multiplies in place over the rep tile."""
                ts = bass.ts(t, NT)
                zps = zp.tile([S, NT], F32, tag="z")
                src0 = xksrc[:, ts].unsqueeze(1)
                if xknext is not None:
                    pws = pw.tile([128, WIDE * NT], F16, tag="pw")
                    pdst = pws[:, :]
                else:
                    pdst = repti[:, 0:WIDE * NT]    # in-place: last reader
                nc.vector.tensor_tensor(
                    pdst.rearrange("p (c n) -> p c n", n=NT),
                    src0.to_broadcast((128, WIDE, NT)),
                    repti[:, 0:WIDE * NT].rearrange("p (c n) -> p c n", n=NT),
                    op=MULT)
                ptiles = {c: pdst[:, c * NT:(c + 1) * NT]
                          for c in range(WIDE)}
                for c in range(WIDE, L12_CH):
                    p1 = psn.tile([128, NT], F16, tag="p1")
                    nc.gpsimd.tensor_tensor(
                        p1[:, :], xksrc[:, ts],
                        repti[:, c * NT:(c + 1) * NT], op=MULT)
                    ptiles[c] = p1[:, :]
                yield
                for c in range(L12_CH):
                    part = 128 if c < L12_CH - 1 else 64
                    nc.tensor.matmul(zps[:, :], ws[:part, bass.ts(c, S)],
                                     ptiles[c][:part, :],
                                     start=(c == 0), stop=(c == L12_CH - 1))
                    if c % 4 == 3:
                        yield
                # epilogue
                if xknext is not None:     # layer 1
                    nc.scalar.activation(
                        xknext[0:64, ts], zps[0:64, :], RELU,
                        bias=bias_s[0:64, zcol_dup:zcol_dup + 1])
                    make_stack(xknext, ts)
                    nc.scalar.activation(
                        tmp01[64:S, (2 + t % 2) * NT:(3 + t % 2) * NT],
                        zps[64:S, :], RELU,
                        bias=bias_s[64:S, zcol_nat:zcol_nat + 1])
                    if t % 2 == 1:
                        # fused d-reduction: direct0+direct1 x two tiles
                        nc.vector.tensor_reduce(
                            o01s[64:S, :].rearrange("p (l q) -> p l q", l=2)
                            [:, :, (t - 1) * 32:(t + 1) * 32],
                            tmp01[64:S, :].rearrange(
                                "p (l b d) -> p l b d", l=2, d=D),
                            axis=AXX, op=ADD)
                else:                      # layer 2
                    nc.scalar.activation(
                        tmp2[:, (t % 2) * NT:(t % 2 + 1) * NT], zps[:, :],
                        RELU, bias=bias_s[:, zcol_nat:zcol_nat + 1])
                    if t % 2 == 1:
                        nc.vector.tensor_reduce(
                            o2s[:, (t - 1) * 32:(t + 1) * 32],
                            tmp2[:, :].rearrange("p (b d) -> p b d", d=D),
                            axis=AXX, op=ADD)
                yield

            pair_tiles = {}

            def stream(t):
                ts = bass.ts(t, NT)
                if t % 2 == 0:
                    tmp01 = tp.tile([S, 4 * NT], F16, tag="tmp01")
                    tmp2 = tp.tile([S, 2 * NT], F16, tag="tmp2")
                    pair_tiles[t // 2] = (tmp01, tmp2)
                tmp01, tmp2 = pair_tiles[t // 2]
                # streaming loads (SP HWDGE ring, one large DMA each)
                Ati = ab.tile([128, L0_CH * NT], F16, tag="A")
                Bti = ab.tile([128, L0_CH * NT], F16, tag="B")
                nc.sync.dma_start(Ati[:, :], Ad[:, bass.ts(t, L0_CH * NT)])
                nc.sync.dma_start(Bti[:, :], Bd[:, bass.ts(t, L0_CH * NT)])
                yield
                # ---- layer 0: one wide multiply, in-place over A --------
                zps = zp.tile([S, NT], F32, tag="z")
                nc.vector.tensor_tensor(Ati[:, :], Ati[:, :], Bti[:, :],
                                        op=MULT)
                repti = rp.tile([128, L12_CH * NT], F16, tag="rep")
                nc.sync.dma_start(repti[:, :],
                                  repd[:, bass.ts(t, L12_CH * NT)])
                yield
                for c in range(L0_CH):
                    part = 128 if c < L0_CH - 1 else L0_LAST
                    nc.tensor.matmul(zps[:, :], w0s[:part, bass.ts(c, S)],
                                     Ati[:part, bass.ts(c, NT)],
                                     start=(c == 0), stop=(c == L0_CH - 1))
                    if c % 4 == 3:
                        yield
                # epilogue L0
                nc.scalar.activation(xk1s[0:64, ts], zps[0:64, :], RELU,
                                     bias=bias_s[0:64, 0:1])
                make_stack(xk1s, ts)
                nc.scalar.activation(tmp01[64:S, (t % 2) * NT:(t % 2 + 1) * NT],
                                     zps[64:S, :], RELU,
                                     bias=bias_s[64:S, 1:2])
                yield
                yield from layer12(t, repti, xk1s, xk2s, w1s, tmp01, tmp2,
                                   2, 3)
                yield from layer12(t, repti, xk2s, None, w2s, tmp01, tmp2,
                                   4, 4)

            pending = list(range(TILES))
            gens = []
            while gens or pending:
                while len(gens) < GROUP and pending:
                    gens.append(stream(pending.pop(0)))
                for gen in list(gens):
                    try:
                        next(gen)
                    except StopIteration:
                        gens.remove(gen)

            nc.scalar.dma_start(y[0:64, :], o01s[64:S, 0:BC])
            nc.scalar.dma_start(y[64:S, :], o01s[64:S, BC:2 * BC])
            nc.scalar.dma_start(y[S:2 * S, :], o2s[:, :])

    _fix_sync_overflow(nc)
    return nc


_NC_CACHE = None


def _get_nc():
    global _NC_CACHE
    if _NC_CACHE is None:
        _NC_CACHE = _build_kernel()
    return _NC_CACHE


# symmetric (h<=k) pair index arrays for layer 0
_HH = np.concatenate([np.full(F0 - h, h, np.int64) for h in range(F0)])
_KK = np.concatenate([np.arange(h, F0) for h in range(F0)])


def _pack_cols(M, nch):
    """[rows<=nch*128, BD] -> [128, TILES*nch*NT]: per column tile t, chunk
    c lives at cols [t*nch*NT + c*NT : ... + NT], partition p = row c*128+p."""
    P = np.zeros((nch * 128, BD), np.float16)
    P[:M.shape[0]] = M
    P = P.reshape(nch, 128, TILES, NT).transpose(1, 2, 0, 3)
    return np.ascontiguousarray(P.reshape(128, TILES * nch * NT))


def _pack_w(Wr, nch):
    """[rows<=nch*128, S] -> [128, nch*S] fp16 chunk-major."""
    P = np.zeros((nch * 128, S), np.float16)
    P[:Wr.shape[0]] = Wr
    return np.ascontiguousarray(
        P.reshape(nch, 128, S).transpose(1, 0, 2).reshape(128, nch * S))


def _prep_shared(w_list, b_list):
    """Weight/bias packing shared by all cores."""
    w0f, w1f, w2f = [np.asarray(w, np.float32) for w in w_list]
    w0sym = w0f[_HH, _KK] + np.where((_HH != _KK)[:, None],
                                     w0f[_KK, _HH], 0.0)   # [780, S]
    w0p = _pack_w(w0sym.astype(np.float16), L0_CH)
    w1p = _pack_w(w1f.reshape(F0 * 64, S).astype(np.float16), L12_CH)
    w2p = _pack_w(w2f.reshape(F0 * 64, S).astype(np.float16), L12_CH)
    b0, b1, b2 = [np.asarray(b, np.float32) for b in b_list]
    biases = np.stack([
        np.concatenate([b0[:64], b0[:64]]), b0,
        np.concatenate([b1[:64], b1[:64]]), b1, b2],
        axis=1).astype(np.float32)          # [128, 5]
    e2 = np.zeros((64, S), np.float16)
    e2[np.arange(64), np.arange(64)] = 1.0
    e2[np.arange(64), 64 + np.arange(64)] = 1.0
    return {"w0d": w0p, "w1d": w1p, "w2d": w2p, "biasd": biases, "e2d": e2}


def _prep_core_inputs(inputs, shared, core):
    """Host-side layout prep for one core's batch slice (data movement
    only: transpose, gather, repeat — no arithmetic)."""
    xs = inputs[core * BC:(core + 1) * BC]          # [BC, F0, D]
    x0t = np.ascontiguousarray(
        xs.transpose(1, 0, 2).reshape(F0, BD)).astype(np.float16)
    A = _pack_cols(x0t[_HH], L0_CH)
    Bm = _pack_cols(x0t[_KK], L0_CH)
    rep = _pack_cols(np.repeat(x0t, 64, axis=0), L12_CH)
    return {"Ad": A, "Bd": Bm, "repd": rep, **shared}


def kernel(inputs, w0, w1, w2, b0, b1, b2, _trace=False):
    inputs = np.asarray(inputs, np.float32)
    shared = _prep_shared((w0, w1, w2), (b0, b1, b2))

    nc = _get_nc()
    in_maps = [_prep_core_inputs(inputs, shared, core)
               for core in range(N_CORES)]
    res = run_bass_kernel_spmd(nc, in_maps, core_ids=list(range(N_CORES)),
                               trace=_trace)
    outs = []
    for core in range(N_CORES):
        yc = res.results[core]["y"]          # [256 s_cat, 256 b]
        outs.append(np.ascontiguousarray(yc.T))
    full = np.concatenate(outs, axis=0)       # [2048, 256]
    if _trace:
        return full, res
    return full
